# revision 4
# baseline (speedup 1.0000x reference)
"""Binarized 3x3 conv block (sign(W) conv + batch-stat BN + ReLU + 2x2 maxpool)
on 8 Trainium2 NeuronCores.

Strategy: data-parallel over batch (4 images/core). The 3x3 stride-1 pad-1
conv is 18 accumulating matmuls per output tile (2 input-channel chunks x 9
taps) with sign(W) as the stationary operand in bf16. Per-channel sum /
sum-of-squares are accumulated on the fly from PSUM (ScalarE accum_out), and
2x2 max- AND min-pools of the raw conv output are computed during PSUM
eviction (maxpool commutes with the monotone BN+ReLU: the answer is
max(relu(s*maxp+b), relu(s*minp+b)) for either sign of s). BN statistics are
combined across the 8 cores with a tiny in-NEFF AllReduce, so the kernel is a
single NEFF launch.
"""

import numpy as np
import ml_dtypes

_NCORES = 8
_B, _C, _H, _W = 32, 256, 56, 56
_BS = _B // _NCORES          # images per core
_PH, _PW = _H + 2, _W + 2    # padded input
_OH, _OW = _H // 2, _W // 2  # pooled output
_EPS = 1e-5
_NSTAT = float(_B * _H * _W)  # elements per channel in the BN stats
_RB = 7                       # row blocks per image (8 output rows each)
_BF16 = ml_dtypes.bfloat16

_CACHE: dict = {}


def _build():
    import concourse.bacc as bacc
    import concourse.mybir as mybir
    import concourse.tile as tile

    f32 = mybir.dt.float32
    bf16 = mybir.dt.bfloat16
    AF = mybir.ActivationFunctionType
    AX = mybir.AxisListType

    nc = bacc.Bacc("TRN2", target_bir_lowering=False, debug=False,
                   num_devices=_NCORES)
    xp_d = nc.dram_tensor("xp", [_BS, _C, _PH, _PW], bf16, kind="ExternalInput")
    w_d = nc.dram_tensor("wt", [2, 128, 9, _C], bf16, kind="ExternalInput")
    g_d = nc.dram_tensor("gm", [2, 128, 1], f32, kind="ExternalInput")
    bt_d = nc.dram_tensor("bt", [2, 128, 1], f32, kind="ExternalInput")
    out_d = nc.dram_tensor("out", [_BS, _C, _OH, _OW], f32, kind="ExternalOutput")

    with tile.TileContext(nc) as tc:
        with (
            tc.tile_pool(name="persist", bufs=1) as keep,
            tc.tile_pool(name="xload", bufs=2) as xpool,
            tc.tile_pool(name="evict", bufs=3) as evp,
            tc.tile_pool(name="acc", bufs=8, space="PSUM") as psp,
            tc.tile_pool(name="dram", bufs=1, space="DRAM") as dpool,
        ):
            # ---- weights / affine params, resident for the whole kernel ----
            w_sb = [keep.tile([128, 9, _C], bf16, tag=f"w{c}", name=f"w{c}") for c in range(2)]
            gm_sb = [keep.tile([128, 1], f32, tag=f"gm{c}", name=f"gm{c}") for c in range(2)]
            bt_sb = [keep.tile([128, 1], f32, tag=f"bt{c}", name=f"bt{c}") for c in range(2)]
            for c in range(2):
                nc.sync.dma_start(w_sb[c][:], w_d[c])
                nc.sync.dma_start(gm_sb[c][:], g_d[c])
                nc.sync.dma_start(bt_sb[c][:], bt_d[c])

            # per-(img,rowblock) stat columns, one pair of tiles per co chunk
            sumc = [keep.tile([128, _BS * _RB], f32, tag=f"sum{c}", name=f"sum{c}") for c in range(2)]
            sqc = [keep.tile([128, _BS * _RB], f32, tag=f"sq{c}", name=f"sq{c}") for c in range(2)]
            # pooled conv outputs (max and min), kept until the apply phase
            pmax = [[keep.tile([128, _OH, _OW], bf16, tag=f"pmax{i}_{c}", name=f"pmax{i}_{c}")
                     for c in range(2)] for i in range(_BS)]
            pmin = [[keep.tile([128, _OH, _OW], bf16, tag=f"pmin{i}_{c}", name=f"pmin{i}_{c}")
                     for c in range(2)] for i in range(_BS)]

            # ---- conv + stats + pooling, one image at a time ----
            for img in range(_BS):
                xt = [xpool.tile([128, _PH, _PW], bf16, tag=f"x{c}", name=f"x{img}_{c}")
                      for c in range(2)]
                for c in range(2):
                    nc.sync.dma_start(xt[c][:], xp_d[img, c * 128:(c + 1) * 128])

                for ch in range(2):            # output channel chunk
                    for rb in range(_RB):      # 8 output rows per block
                        ps = psp.tile([128, 8, _W], f32, tag="acc", name=f"acc{img}_{ch}_{rb}")
                        k = 0
                        for cic in range(2):   # input channel chunk
                            for kh in range(3):
                                for kw in range(3):
                                    lhsT = w_sb[cic][:, kh * 3 + kw,
                                                     ch * 128:(ch + 1) * 128]
                                    rhs = xt[cic][:, rb * 8 + kh: rb * 8 + kh + 8,
                                                  kw: kw + _W]
                                    nc.tensor.matmul(ps[:], lhsT, rhs,
                                                     start=(k == 0),
                                                     stop=(k == 17))
                                    k += 1
                        col = img * _RB + rb
                        ybf = evp.tile([128, 8, _W], bf16, tag="ybf", name=f"ybf{col}_{ch}")
                        nc.scalar.activation(
                            ybf[:], ps[:], AF.Copy,
                            accum_out=sumc[ch][:, col:col + 1])
                        ysq = evp.tile([128, 8, _W], bf16, tag="ysq", name=f"ysq{col}_{ch}")
                        nc.scalar.activation(
                            ysq[:], ps[:], AF.Square,
                            accum_out=sqc[ch][:, col:col + 1])
                        # 2x2 pooling of the raw conv tile (max and min)
                        a = ybf[:, 0:8:2, 0:_W:2]
                        b = ybf[:, 0:8:2, 1:_W:2]
                        cc = ybf[:, 1:8:2, 0:_W:2]
                        d = ybf[:, 1:8:2, 1:_W:2]
                        t1 = evp.tile([128, 4, _OW], bf16, tag="t1", name=f"t1_{col}_{ch}")
                        t2 = evp.tile([128, 4, _OW], bf16, tag="t2", name=f"t2_{col}_{ch}")
                        nc.vector.tensor_max(t1[:], a, b)
                        nc.vector.tensor_max(t2[:], cc, d)
                        nc.vector.tensor_max(
                            pmax[img][ch][:, rb * 4:(rb + 1) * 4, :], t1[:], t2[:])
                        t3 = evp.tile([128, 4, _OW], bf16, tag="t3", name=f"t3_{col}_{ch}")
                        t4 = evp.tile([128, 4, _OW], bf16, tag="t4", name=f"t4_{col}_{ch}")
                        nc.vector.tensor_tensor(t3[:], a, b, op=mybir.AluOpType.min)
                        nc.vector.tensor_tensor(t4[:], cc, d, op=mybir.AluOpType.min)
                        nc.vector.tensor_tensor(
                            pmin[img][ch][:, rb * 4:(rb + 1) * 4, :], t3[:], t4[:],
                            op=mybir.AluOpType.min)

            # ---- global BN statistics: 8-core AllReduce of [128,4] sums ----
            stats = keep.tile([128, 4], f32, tag="stats", name="stats")
            for ch in range(2):
                nc.vector.reduce_sum(stats[:, 2 * ch:2 * ch + 1], sumc[ch][:],
                                     axis=AX.X)
                nc.vector.reduce_sum(stats[:, 2 * ch + 1:2 * ch + 2], sqc[ch][:],
                                     axis=AX.X)
            cc_in = dpool.tile([128, 4], f32, tag="ccin", name="ccin")
            cc_out = dpool.tile([128, 4], f32, tag="ccout", name="ccout")
            nc.gpsimd.dma_start(cc_in[:], stats[:])
            nc.gpsimd.collective_compute(
                "AllReduce", mybir.AluOpType.add,
                replica_groups=[list(range(_NCORES))],
                ins=[cc_in.opt()], outs=[cc_out.opt()])
            gstats = keep.tile([128, 4], f32, tag="gstats", name="gstats")
            nc.gpsimd.dma_start(gstats[:], cc_out[:])

            # ---- per-channel scale/bias: s = gamma*rsqrt(var+eps), b' = beta-mean*s
            eps = keep.tile([128, 1], f32, tag="eps", name="eps")
            nc.vector.memset(eps[:], _EPS)
            scl, bia = [], []
            for ch in range(2):
                mean = keep.tile([128, 1], f32, tag=f"mean{ch}", name=f"mean{ch}")
                q = keep.tile([128, 1], f32, tag=f"q{ch}", name=f"q{ch}")
                m2 = keep.tile([128, 1], f32, tag=f"m2{ch}", name=f"m2{ch}")
                var = keep.tile([128, 1], f32, tag=f"var{ch}", name=f"var{ch}")
                sd = keep.tile([128, 1], f32, tag=f"sd{ch}", name=f"sd{ch}")
                inv = keep.tile([128, 1], f32, tag=f"inv{ch}", name=f"inv{ch}")
                s = keep.tile([128, 1], f32, tag=f"s{ch}", name=f"s{ch}")
                ms = keep.tile([128, 1], f32, tag=f"ms{ch}", name=f"ms{ch}")
                bb = keep.tile([128, 1], f32, tag=f"bb{ch}", name=f"bb{ch}")
                nc.scalar.mul(mean[:], gstats[:, 2 * ch:2 * ch + 1], 1.0 / _NSTAT)
                nc.scalar.mul(q[:], gstats[:, 2 * ch + 1:2 * ch + 2], 1.0 / _NSTAT)
                nc.vector.tensor_mul(m2[:], mean[:], mean[:])
                nc.vector.tensor_sub(var[:], q[:], m2[:])
                nc.scalar.activation(sd[:], var[:], AF.Sqrt, bias=eps[:])
                nc.vector.reciprocal(inv[:], sd[:])
                nc.vector.tensor_mul(s[:], gm_sb[ch][:], inv[:])
                nc.vector.tensor_mul(ms[:], mean[:], s[:])
                nc.vector.tensor_sub(bb[:], bt_sb[ch][:], ms[:])
                scl.append(s)
                bia.append(bb)

            # ---- apply BN+ReLU to both pooled candidates, combine, store ----
            for img in range(_BS):
                for ch in range(2):
                    r1 = evp.tile([128, _OH, _OW], f32, tag="r1", name=f"r1_{img}_{ch}")
                    r2 = evp.tile([128, _OH, _OW], f32, tag="r2", name=f"r2_{img}_{ch}")
                    nc.scalar.activation(r1[:], pmax[img][ch][:], AF.Relu,
                                         bias=bia[ch][:], scale=scl[ch][:])
                    nc.scalar.activation(r2[:], pmin[img][ch][:], AF.Relu,
                                         bias=bia[ch][:], scale=scl[ch][:])
                    res = evp.tile([128, _OH, _OW], f32, tag="res", name=f"res_{img}_{ch}")
                    nc.vector.tensor_max(res[:], r1[:], r2[:])
                    nc.sync.dma_start(out_d[img, ch * 128:(ch + 1) * 128], res[:])

    nc.compile()
    return nc


def _prep_inputs(x, W, gamma, beta):
    x = np.asarray(x, dtype=np.float32)
    W = np.asarray(W, dtype=np.float32)
    gamma = np.asarray(gamma, dtype=np.float32)
    beta = np.asarray(beta, dtype=np.float32)

    # sign-binarized weights, laid out [ci_chunk, ci_in_chunk, tap, co] bf16
    wb = np.sign(W)                                    # [co, ci, 3, 3]
    wt = wb.transpose(1, 2, 3, 0).reshape(2, 128, 9, _C)
    wt = np.ascontiguousarray(wt).astype(_BF16)

    # zero-padded, bf16-cast input
    xp = np.zeros((_B, _C, _PH, _PW), dtype=_BF16)
    xp[:, :, 1:_H + 1, 1:_W + 1] = x.astype(_BF16)

    gm = np.ascontiguousarray(gamma.reshape(2, 128, 1))
    bt = np.ascontiguousarray(beta.reshape(2, 128, 1))

    in_maps = []
    for core in range(_NCORES):
        in_maps.append({
            "xp": np.ascontiguousarray(xp[core * _BS:(core + 1) * _BS]),
            "wt": wt,
            "gm": gm,
            "bt": bt,
        })
    return in_maps


def _run(x, W, gamma, beta, trace=False):
    from concourse.bass_utils import run_bass_kernel_spmd

    if "nc" not in _CACHE:
        _CACHE["nc"] = _build()
    nc = _CACHE["nc"]
    in_maps = _prep_inputs(x, W, gamma, beta)
    res = run_bass_kernel_spmd(nc, in_maps, core_ids=list(range(_NCORES)),
                               trace=trace)
    out = np.concatenate([res.results[c]["out"] for c in range(_NCORES)], axis=0)
    return np.ascontiguousarray(out.astype(np.float32)), res


def kernel(x, W, gamma, beta):
    out, _ = _run(x, W, gamma, beta, trace=False)
    return out


# revision 6
# speedup vs baseline: 1.0717x; 1.0717x over previous
"""Binarized 3x3 conv block (sign(W) conv + batch-stat BN + ReLU + 2x2 maxpool)
on 8 Trainium2 NeuronCores.

Strategy: data-parallel over batch (4 images/core). The 3x3 stride-1 pad-1
conv is 18 accumulating matmuls per output row-block (2 input-channel chunks
x 9 taps) with sign(W) as the stationary operand in bf16. Per-channel sum /
sum-of-squares are accumulated during PSUM eviction (ScalarE accum_out), and
2x2 max- AND min-pools of the raw conv output are computed at the same time
(maxpool commutes with the monotone BN+ReLU: the answer is
relu(max(s*maxp, s*minp) + b) for either sign of s). The output-channel
chunks are processed serially so chunk 0's cross-core stats AllReduce and
BN-apply hide under chunk 1's conv; only chunk 1's collective + apply are
exposed at the tail.
"""

import numpy as np
import ml_dtypes

_NCORES = 8
_B, _C, _H, _W = 32, 256, 56, 56
_BS = _B // _NCORES          # images per core
_PH, _PW = _H + 2, _W + 2    # padded input
_OH, _OW = _H // 2, _W // 2  # pooled output
_EPS = 1e-5
_NSTAT = float(_B * _H * _W)  # elements per channel in the BN stats
_RB = 7                       # row blocks per image (8 output rows each)
_BF16 = ml_dtypes.bfloat16

_CACHE: dict = {}


def _build():
    import concourse.bacc as bacc
    import concourse.mybir as mybir
    import concourse.tile as tile

    f32 = mybir.dt.float32
    bf16 = mybir.dt.bfloat16
    AF = mybir.ActivationFunctionType
    AX = mybir.AxisListType
    OP = mybir.AluOpType

    nc = bacc.Bacc("TRN2", target_bir_lowering=False, debug=False,
                   num_devices=_NCORES)
    xp_d = nc.dram_tensor("xp", [_BS, _C, _PH, _PW], bf16, kind="ExternalInput")
    w_d = nc.dram_tensor("wt", [2, 128, 9, _C], bf16, kind="ExternalInput")
    g_d = nc.dram_tensor("gm", [2, 128, 1], f32, kind="ExternalInput")
    bt_d = nc.dram_tensor("bt", [2, 128, 1], f32, kind="ExternalInput")
    out_d = nc.dram_tensor("out", [_BS, _C, _OH, _OW], f32, kind="ExternalOutput")

    with tile.TileContext(nc) as tc:
        with (
            tc.tile_pool(name="persist", bufs=1) as keep,
            tc.tile_pool(name="evict", bufs=3) as evp,
            tc.tile_pool(name="acc", bufs=8, space="PSUM") as psp,
            tc.tile_pool(name="dram", bufs=1, space="DRAM") as dpool,
        ):
            # ---- weights first (vector queue), then all images (sync queue)
            w_sb = [keep.tile([128, 9, _C], bf16, tag=f"w{c}", name=f"w{c}")
                    for c in range(2)]
            for c in range(2):
                nc.scalar.dma_start(w_sb[c][:], w_d[c])
            xt = [[keep.tile([128, _PH, _PW], bf16, tag=f"x{i}_{c}",
                             name=f"x{i}_{c}") for c in range(2)]
                  for i in range(_BS)]
            for i in range(_BS):
                for c in range(2):
                    nc.sync.dma_start(xt[i][c][:], xp_d[i, c * 128:(c + 1) * 128])
            gm_sb = [keep.tile([128, 1], f32, tag=f"gm{c}", name=f"gm{c}")
                     for c in range(2)]
            bt_sb = [keep.tile([128, 1], f32, tag=f"bt{c}", name=f"bt{c}")
                     for c in range(2)]
            for c in range(2):
                nc.scalar.dma_start(gm_sb[c][:], g_d[c])
                nc.scalar.dma_start(bt_sb[c][:], bt_d[c])
            eps = keep.tile([128, 1], f32, tag="eps", name="eps")
            nc.gpsimd.memset(eps[:], _EPS)

            sumc = [keep.tile([128, _BS * _RB], f32, tag=f"sum{c}", name=f"sum{c}")
                    for c in range(2)]
            sqc = [keep.tile([128, _BS * _RB], f32, tag=f"sq{c}", name=f"sq{c}")
                   for c in range(2)]
            pmax = [[keep.tile([128, _OH, _OW], bf16, tag=f"pmax{i}_{c}",
                               name=f"pmax{i}_{c}") for c in range(2)]
                    for i in range(_BS)]
            pmin = [[keep.tile([128, _OH, _OW], bf16, tag=f"pmin{i}_{c}",
                               name=f"pmin{i}_{c}") for c in range(2)]
                    for i in range(_BS)]
            gstats = [keep.tile([128, 2], f32, tag=f"gstats{c}", name=f"gstats{c}")
                      for c in range(2)]

            # ---- conv + on-the-fly stats/pools, then the chunk's AllReduce ----
            for ch in range(2):            # output channel chunk (serialized)
                for img in range(_BS):
                    for rb in range(_RB):  # 8 output rows per block
                        ps = psp.tile([128, 8, _W], f32, tag="acc",
                                      name=f"acc{ch}_{img}_{rb}")
                        k = 0
                        for cic in range(2):
                            for kh in range(3):
                                for kw in range(3):
                                    lhsT = w_sb[cic][:, kh * 3 + kw,
                                                     ch * 128:(ch + 1) * 128]
                                    rhs = xt[img][cic][:,
                                                       rb * 8 + kh: rb * 8 + kh + 8,
                                                       kw: kw + _W]
                                    nc.tensor.matmul(ps[:], lhsT, rhs,
                                                     start=(k == 0),
                                                     stop=(k == 17))
                                    k += 1
                        col = img * _RB + rb
                        ybf = evp.tile([128, 8, _W], bf16, tag="ybf",
                                       name=f"ybf{ch}_{col}")
                        nc.scalar.activation(
                            ybf[:], ps[:], AF.Copy,
                            accum_out=sumc[ch][:, col:col + 1])
                        ysq = evp.tile([128, 8, _W], bf16, tag="ysq",
                                       name=f"ysq{ch}_{col}")
                        nc.scalar.activation(
                            ysq[:], ps[:], AF.Square,
                            accum_out=sqc[ch][:, col:col + 1])
                        a = ybf[:, 0:8:2, 0:_W:2]
                        b = ybf[:, 0:8:2, 1:_W:2]
                        cc = ybf[:, 1:8:2, 0:_W:2]
                        d = ybf[:, 1:8:2, 1:_W:2]
                        t1 = evp.tile([128, 4, _OW], bf16, tag="t1",
                                      name=f"t1_{ch}_{col}")
                        t2 = evp.tile([128, 4, _OW], bf16, tag="t2",
                                      name=f"t2_{ch}_{col}")
                        nc.vector.tensor_max(t1[:], a, b)
                        nc.vector.tensor_max(t2[:], cc, d)
                        nc.vector.tensor_max(
                            pmax[img][ch][:, rb * 4:(rb + 1) * 4, :], t1[:], t2[:])
                        t3 = evp.tile([128, 4, _OW], bf16, tag="t3",
                                      name=f"t3_{ch}_{col}")
                        t4 = evp.tile([128, 4, _OW], bf16, tag="t4",
                                      name=f"t4_{ch}_{col}")
                        nc.vector.tensor_tensor(t3[:], a, b, op=OP.min)
                        nc.vector.tensor_tensor(t4[:], cc, d, op=OP.min)
                        nc.vector.tensor_tensor(
                            pmin[img][ch][:, rb * 4:(rb + 1) * 4, :], t3[:], t4[:],
                            op=OP.min)

                # chunk's global stats: [128,2] AllReduce across the 8 cores
                stats = keep.tile([128, 2], f32, tag=f"stats{ch}",
                                  name=f"stats{ch}")
                nc.vector.reduce_sum(stats[:, 0:1], sumc[ch][:], axis=AX.X)
                nc.vector.reduce_sum(stats[:, 1:2], sqc[ch][:], axis=AX.X)
                cc_in = dpool.tile([128, 2], f32, tag=f"ccin{ch}",
                                   name=f"ccin{ch}")
                cc_out = dpool.tile([128, 2], f32, tag=f"ccout{ch}",
                                    name=f"ccout{ch}")
                nc.gpsimd.dma_start(cc_in[:], stats[:])
                nc.gpsimd.collective_compute(
                    "AllReduce", OP.add,
                    replica_groups=[list(range(_NCORES))],
                    ins=[cc_in.opt()], outs=[cc_out.opt()])
                nc.gpsimd.dma_start(gstats[ch][:], cc_out[:])

            # ---- per-chunk scale/bias + BN/ReLU apply + store ----
            # (emitted after both conv chunks: chunk 0's instructions have no
            # outstanding deps by the time engines reach them, so they fill
            # the window while chunk 1's collective runs)
            for ch in range(2):
                meanq = keep.tile([128, 2], f32, tag=f"meanq{ch}",
                                  name=f"meanq{ch}")
                m2 = keep.tile([128, 1], f32, tag=f"m2{ch}", name=f"m2{ch}")
                var = keep.tile([128, 1], f32, tag=f"var{ch}", name=f"var{ch}")
                sd = keep.tile([128, 1], f32, tag=f"sd{ch}", name=f"sd{ch}")
                inv = keep.tile([128, 1], f32, tag=f"inv{ch}", name=f"inv{ch}")
                s = keep.tile([128, 1], f32, tag=f"s{ch}", name=f"s{ch}")
                ms = keep.tile([128, 1], f32, tag=f"ms{ch}", name=f"ms{ch}")
                bb = keep.tile([128, 1], f32, tag=f"bb{ch}", name=f"bb{ch}")
                nc.scalar.mul(meanq[:], gstats[ch][:], 1.0 / _NSTAT)
                nc.vector.tensor_mul(m2[:], meanq[:, 0:1], meanq[:, 0:1])
                nc.vector.tensor_sub(var[:], meanq[:, 1:2], m2[:])
                nc.scalar.activation(sd[:], var[:], AF.Sqrt, bias=eps[:])
                nc.vector.reciprocal(inv[:], sd[:])
                nc.vector.tensor_mul(s[:], gm_sb[ch][:], inv[:])
                nc.vector.tensor_mul(ms[:], meanq[:, 0:1], s[:])
                nc.vector.tensor_sub(bb[:], bt_sb[ch][:], ms[:])

                for img in range(_BS):
                    u = evp.tile([128, _OH, _OW], f32, tag="u",
                                 name=f"u{ch}_{img}")
                    m = evp.tile([128, _OH, _OW], f32, tag="m",
                                 name=f"m{ch}_{img}")
                    nc.vector.tensor_scalar_mul(u[:], pmax[img][ch][:], s[:])
                    nc.vector.scalar_tensor_tensor(
                        m[:], pmin[img][ch][:], s[:], u[:],
                        op0=OP.mult, op1=OP.max)
                    res = evp.tile([128, _OH, _OW], f32, tag="res",
                                   name=f"res{ch}_{img}")
                    nc.scalar.activation(res[:], m[:], AF.Relu, bias=bb[:])
                    nc.sync.dma_start(out_d[img, ch * 128:(ch + 1) * 128], res[:])

    nc.compile()
    return nc


def _prep_inputs(x, W, gamma, beta):
    x = np.asarray(x, dtype=np.float32)
    W = np.asarray(W, dtype=np.float32)
    gamma = np.asarray(gamma, dtype=np.float32)
    beta = np.asarray(beta, dtype=np.float32)

    # sign-binarized weights, laid out [ci_chunk, ci_in_chunk, tap, co] bf16
    wb = np.sign(W)                                    # [co, ci, 3, 3]
    wt = wb.transpose(1, 2, 3, 0).reshape(2, 128, 9, _C)
    wt = np.ascontiguousarray(wt).astype(_BF16)

    # zero-padded, bf16-cast input
    xp = np.zeros((_B, _C, _PH, _PW), dtype=_BF16)
    xp[:, :, 1:_H + 1, 1:_W + 1] = x.astype(_BF16)

    gm = np.ascontiguousarray(gamma.reshape(2, 128, 1))
    bt = np.ascontiguousarray(beta.reshape(2, 128, 1))

    in_maps = []
    for core in range(_NCORES):
        in_maps.append({
            "xp": np.ascontiguousarray(xp[core * _BS:(core + 1) * _BS]),
            "wt": wt,
            "gm": gm,
            "bt": bt,
        })
    return in_maps


def _run(x, W, gamma, beta, trace=False):
    from concourse.bass_utils import run_bass_kernel_spmd

    if "nc" not in _CACHE:
        _CACHE["nc"] = _build()
    nc = _CACHE["nc"]
    in_maps = _prep_inputs(x, W, gamma, beta)
    res = run_bass_kernel_spmd(nc, in_maps, core_ids=list(range(_NCORES)),
                               trace=trace)
    out = np.concatenate([res.results[c]["out"] for c in range(_NCORES)], axis=0)
    return np.ascontiguousarray(out.astype(np.float32)), res


def kernel(x, W, gamma, beta):
    out, _ = _run(x, W, gamma, beta, trace=False)
    return out
